# revision 1
# baseline (speedup 1.0000x reference)
"""BiasedAxialAttention (row-attention path) distributed over 8 TRN2 NeuronCores.

Sharding: outer (non-attended) L axis "n" (= p axis 1 after the reference's
permute, = pair axis 2) split into 8 slices of 48 rows.

Per-core dataflow (all shapes hardcoded for B=1, L=384, D=128, H=4, DH=32):
  phase 1: LN(x_qkv) -> q,k projections emitted directly in the shuffled
           [(k*4+s), i] layout via scatter-column weights (4 accumulating
           matmuls per 4-n group), v projection in [j, (h,d)] layout,
           b = bias @ Wb^T via PE-transposed bias tiles,
           logits[i,j,h] accumulated over 12 groups at K=128.
  phase 2: ReduceScatter(logits over i) -> +b -> softmax(j) -> AllGather.
           gate path (LN + Wg + sigmoid) overlaps the collectives.
  phase 3: transpose attn -> AV (per-head 32-row strips of one PSUM tile),
           gate multiply (+folded cv via softmax-sums-to-1), Wo, +bo, DMA out.
"""

import math

import numpy as np
import ml_dtypes

H, DH, D, L = 4, 32, 128, 384
NCORES = 8
R = L // NCORES  # 48
SCALING = 1.0 / math.sqrt(DH)
KSCALE = 1.0 / math.sqrt(L)
EPS = 1e-5
NG = R // 4  # 12 groups of 4 n-rows

_CACHE = {}


def _build_graph():
    import concourse.bass as bass
    import concourse.tile as tile
    from concourse import bacc, mybir

    f32 = mybir.dt.float32
    bf16 = mybir.dt.bfloat16
    Exp = mybir.ActivationFunctionType.Exp
    Identity = mybir.ActivationFunctionType.Identity
    Sigmoid = mybir.ActivationFunctionType.Sigmoid
    Sqrt = mybir.ActivationFunctionType.Sqrt
    sub = mybir.AluOpType.subtract
    mult = mybir.AluOpType.mult
    add = mybir.AluOpType.add

    nc = bacc.Bacc(
        "TRN2", target_bir_lowering=False, debug=False, num_devices=NCORES
    )

    # ---- external params (per-core shards + shared prepped weights) ----
    x_qkv = nc.declare_dram_parameter("x_qkv", [R, L, D], f32, isOutput=False)
    x_gate = nc.declare_dram_parameter("x_gate", [R, L, D], f32, isOutput=False)
    bias_c = nc.declare_dram_parameter("bias_c", [R, L, D], f32, isOutput=False)
    wq_scat = nc.declare_dram_parameter("wq_scat", [16, D, D], bf16, isOutput=False)
    wk_scat = nc.declare_dram_parameter("wk_scat", [16, D, D], bf16, isOutput=False)
    wv_t = nc.declare_dram_parameter("wv_t", [D, D], bf16, isOutput=False)
    wg_t = nc.declare_dram_parameter("wg_t", [D, D], bf16, isOutput=False)
    wo_t = nc.declare_dram_parameter("wo_t", [D, D], bf16, isOutput=False)
    wb_t = nc.declare_dram_parameter("wb_t", [D, H], f32, isOutput=False)
    cq_sh = nc.declare_dram_parameter("cq_sh", [H, D], f32, isOutput=False)
    ck_sh = nc.declare_dram_parameter("ck_sh", [H, D], f32, isOutput=False)
    cg_v = nc.declare_dram_parameter("cg_v", [D], f32, isOutput=False)
    cv_v = nc.declare_dram_parameter("cv_v", [D], f32, isOutput=False)
    bo_v = nc.declare_dram_parameter("bo_v", [D], f32, isOutput=False)
    out_p = nc.declare_dram_parameter("out", [R, L, D], f32, isOutput=True)

    # ---- internal DRAM (collective bounces; outs must be Shared) ----
    logits_dram = nc.dram_tensor("logits_dram", [L, H, L], bf16)
    rs_out = nc.dram_tensor("rs_out", [R, H, L], bf16)
    attn_bounce = nc.dram_tensor("attn_bounce", [R, H, L], bf16)
    attn_full = nc.dram_tensor("attn_full", [L, H, L], bf16, addr_space="Shared")
    groups = [list(range(NCORES))]

    with tile.TileContext(nc) as tc:
        from contextlib import ExitStack

        with ExitStack() as top:
            consts = top.enter_context(tc.tile_pool(name="consts", bufs=1))

            # constant tiles
            id_bf = consts.tile([D, D], bf16)
            id_f32 = consts.tile([D, D], f32)
            wqs_sb = consts.tile([D, 16, D], bf16)   # [d, (h,s), P]
            wks_sb = consts.tile([D, 16, D], bf16)
            wv_sb = consts.tile([D, D], bf16)
            wg_sb = consts.tile([D, D], bf16)
            wo_sb = consts.tile([D, D], bf16)
            wb_sb = consts.tile([D, H], f32)
            cq_sb = consts.tile([D, H], f32)         # per-partition bias, col h
            ck_sb = consts.tile([D, H], f32)
            cg_sb = consts.tile([D, 1], f32)
            cv_sb = consts.tile([D, 1], f32)
            bo_bc = consts.tile([D, D], f32)         # bo broadcast along partitions
            eps_sb = consts.tile([D, 1], f32)

            from concourse.masks import make_identity

            make_identity(nc, id_bf)
            make_identity(nc, id_f32)
            nc.sync.dma_start(out=wqs_sb, in_=wq_scat.ap().rearrange("s d p -> d s p"))
            nc.sync.dma_start(out=wks_sb, in_=wk_scat.ap().rearrange("s d p -> d s p"))
            nc.sync.dma_start(out=wv_sb, in_=wv_t[:, :])
            nc.sync.dma_start(out=wg_sb, in_=wg_t[:, :])
            nc.sync.dma_start(out=wo_sb, in_=wo_t[:, :])
            nc.sync.dma_start(out=wb_sb, in_=wb_t[:, :])
            nc.sync.dma_start(out=cq_sb, in_=cq_sh.ap().rearrange("h d -> d h"))
            nc.sync.dma_start(out=ck_sb, in_=ck_sh.ap().rearrange("h d -> d h"))
            nc.sync.dma_start(out=cg_sb, in_=cg_v.ap().unsqueeze(1))
            nc.sync.dma_start(out=cv_sb, in_=cv_v.ap().unsqueeze(1))
            nc.sync.dma_start(
                out=bo_bc, in_=bo_v.ap().unsqueeze(0).broadcast_to((D, D))
            )
            nc.vector.memset(eps_sb, EPS)

            # persistent stores
            stores = top.enter_context(tc.tile_pool(name="stores", bufs=1))
            v_st = stores.tile([D, 3, R, D], bf16)      # [j, jc, y, (h,d)]
            g_st = stores.tile([D, R, L], bf16)         # [(h,d), y, x]
            b_st = stores.tile([D, 3, H, R], f32)       # [j-part, jc, h, i]

            # ---------------- phase 1: QKV + b + logits ----------------
            qk_ctx = ExitStack()
            qk_st = qk_ctx.enter_context(tc.tile_pool(name="qk_st", bufs=1))
            qsh = qk_st.tile([D, H, NG, L], bf16)   # [(k,s), h, g, i]
            ksh = qk_st.tile([D, H, NG, L], bf16)
            with ExitStack() as ph1:
                xin_p = ph1.enter_context(tc.tile_pool(name="xin", bufs=4))
                st_p = ph1.enter_context(tc.tile_pool(name="stats", bufs=6))
                xh_p = ph1.enter_context(tc.tile_pool(name="xh", bufs=4))
                xt_p = ph1.enter_context(
                    tc.tile_pool(name="xt", bufs=2, space="PSUM")
                )
                xts_p = ph1.enter_context(tc.tile_pool(name="xts", bufs=6))
                vps_p = ph1.enter_context(
                    tc.tile_pool(name="vps", bufs=1, space="PSUM")
                )
                slab_p = ph1.enter_context(
                    tc.tile_pool(name="slab", bufs=5, space="PSUM")
                )

                def load_x4(src_dram, g):
                    xin4 = xin_p.tile([D, 4, 3, D], f32, tag="xin")
                    nc.sync.dma_start(
                        out=xin4.rearrange("p s t d -> p (s t d)"),
                        in_=bass.AP(
                            tensor=src_dram.ap().tensor,
                            offset=4 * g * L * D,
                            ap=[[3 * D, D], [L * D, 4], [1, 3 * D]],
                        ),
                    )
                    return xin4

                def ln_to_xhatT(xin, engine_idx):
                    """LN rows of xin [128, 3, 128]; return [d, 384] bf16 transposed."""
                    stt = st_p.tile([D, 3, 6], f32, tag="st")
                    mv = st_p.tile([D, 3, 2], f32, tag="mv")
                    for t in range(3):
                        nc.vector.bn_stats(out=stt[:, t, :], in_=xin[:, t, :])
                        nc.vector.bn_aggr(out=mv[:, t, :], in_=stt[:, t, :])
                    sd = st_p.tile([D, 3], f32, tag="sd")
                    nc.scalar.activation(
                        out=sd, in_=mv[:, :, 1], func=Sqrt, bias=eps_sb, scale=1.0
                    )
                    istd = st_p.tile([D, 3], f32, tag="istd")
                    nc.vector.reciprocal(out=istd, in_=sd)
                    xh = xh_p.tile([D, 3, D], bf16, tag="xh")
                    for t in range(3):
                        nc.gpsimd.tensor_scalar(
                            out=xh[:, t, :],
                            in0=xin[:, t, :],
                            scalar1=mv[:, t, 0:1],
                            scalar2=istd[:, t : t + 1],
                            op0=sub,
                            op1=mult,
                        )
                    xt = xt_p.tile([D, L], bf16, tag="xt")
                    for t in range(3):
                        nc.tensor.transpose(
                            out=xt[:, t * D : (t + 1) * D], in_=xh[:, t, :],
                            identity=id_bf,
                        )
                    xts = xts_p.tile([D, L], bf16, tag="xts")
                    if engine_idx % 2 == 0:
                        nc.vector.tensor_copy(out=xts, in_=xt)
                    else:
                        nc.scalar.activation(out=xts, in_=xt, func=Identity)
                    return xts

                for g in range(NG):
                    xts_g = []
                    xin4 = load_x4(x_qkv, g)
                    psq = [slab_p.tile([D, L], f32, tag="slab", name=f"psq_{g}_{h}") for h in range(H)]
                    for s in range(4):
                        n = 4 * g + s
                        xts = ln_to_xhatT(xin4[:, s], n)
                        xts_g.append(xts)
                        # v projection: [j-chunk, (h,d)] x3 into one psum bank
                        vps = vps_p.tile([D, 3, D], f32, tag="vps")
                        for jc in range(3):
                            nc.tensor.matmul(
                                vps[:, jc, :],
                                xts[:, jc * D : (jc + 1) * D],
                                wv_sb,
                                start=True,
                                stop=True,
                            )
                        nc.vector.tensor_copy(out=v_st[:, :, n, :], in_=vps)
                        # q scattered projections accumulate into 4 head slabs
                        for h in range(H):
                            nc.tensor.matmul(
                                psq[h],
                                wqs_sb[:, h * 4 + s, :],
                                xts,
                                start=(s == 0),
                                stop=(s == 3),
                            )
                    for h in range(H):
                        nc.scalar.activation(
                            out=qsh[:, h, g, :], in_=psq[h], func=Identity,
                            bias=cq_sb[:, h : h + 1], scale=1.0,
                        )
                    psk = [slab_p.tile([D, L], f32, tag="slab", name=f"psk_{g}_{h}") for h in range(H)]
                    for s in range(4):
                        for h in range(H):
                            nc.tensor.matmul(
                                psk[h],
                                wks_sb[:, h * 4 + s, :],
                                xts_g[s],
                                start=(s == 0),
                                stop=(s == 3),
                            )
                    for h in range(H):
                        nc.scalar.activation(
                            out=ksh[:, h, g, :], in_=psk[h], func=Identity,
                            bias=ck_sb[:, h : h + 1], scale=1.0,
                        )



            def gate_rows(scope, pfx, y0, y1):
                xin_p = scope.enter_context(
                    tc.tile_pool(name=pfx + "xin2", bufs=3)
                )
                st_p = scope.enter_context(
                    tc.tile_pool(name=pfx + "stats2", bufs=6)
                )
                xh_p = scope.enter_context(
                    tc.tile_pool(name=pfx + "xh2", bufs=4)
                )
                xt_p = scope.enter_context(
                    tc.tile_pool(name=pfx + "xt2", bufs=2, space="PSUM")
                )
                xts_p = scope.enter_context(
                    tc.tile_pool(name=pfx + "xts2", bufs=4)
                )
                gp_p = scope.enter_context(
                    tc.tile_pool(name=pfx + "gp", bufs=2, space="PSUM")
                )
                for y in range(y0, y1):
                    if y % 4 == 0:
                        xin4g = xin_p.tile([D, 4, 3, D], f32, tag="xin",
                                           name=f"{pfx}xin4g_{y}")
                        nc.sync.dma_start(
                            out=xin4g.rearrange("p s t d -> p (s t d)"),
                            in_=bass.AP(
                                tensor=x_gate.ap().tensor,
                                offset=y * L * D,
                                ap=[[3 * D, D], [L * D, 4], [1, 3 * D]],
                            ),
                        )
                    xin = xin4g[:, y % 4]
                    stt = st_p.tile([D, 3, 6], f32, tag="st")
                    mv = st_p.tile([D, 3, 2], f32, tag="mv")
                    for t in range(3):
                        nc.vector.bn_stats(out=stt[:, t, :], in_=xin[:, t, :])
                        nc.vector.bn_aggr(out=mv[:, t, :], in_=stt[:, t, :])
                    sd = st_p.tile([D, 3], f32, tag="sd")
                    nc.scalar.activation(
                        out=sd, in_=mv[:, :, 1], func=Sqrt, bias=eps_sb, scale=1.0
                    )
                    istd = st_p.tile([D, 3], f32, tag="istd")
                    nc.vector.reciprocal(out=istd, in_=sd)
                    xh = xh_p.tile([D, 3, D], bf16, tag="xh")
                    for t in range(3):
                        nc.gpsimd.tensor_scalar(
                            out=xh[:, t, :],
                            in0=xin[:, t, :],
                            scalar1=mv[:, t, 0:1],
                            scalar2=istd[:, t : t + 1],
                            op0=sub,
                            op1=mult,
                        )
                    xt = xt_p.tile([D, L], bf16, tag="xt")
                    for t in range(3):
                        nc.tensor.transpose(
                            out=xt[:, t * D : (t + 1) * D], in_=xh[:, t, :],
                            identity=id_bf,
                        )
                    xts = xts_p.tile([D, L], bf16, tag="xts")
                    nc.vector.tensor_copy(out=xts, in_=xt)
                    gp = gp_p.tile([D, L], f32, tag="gp")
                    nc.tensor.matmul(gp, wg_sb, xts, start=True, stop=True)
                    nc.scalar.activation(
                        out=g_st[:, y, :], in_=gp, func=Sigmoid, bias=cg_sb,
                        scale=1.0,
                    )

            # logits: [i-chunk, j] per head, K=128 over 12 groups
            with ExitStack() as phl:
                lg_p = phl.enter_context(
                    tc.tile_pool(name="lgp", bufs=2, space="PSUM")
                )
                ls_p = phl.enter_context(tc.tile_pool(name="lsb", bufs=3))
                bt_p = phl.enter_context(tc.tile_pool(name="bt", bufs=2))
                btp_p = phl.enter_context(
                    tc.tile_pool(name="btp", bufs=1, space="PSUM")
                )
                bps_p = phl.enter_context(
                    tc.tile_pool(name="bps", bufs=1, space="PSUM")
                )
                gate_rows(phl, "a_", 0, 24)
                for g in range(NG):
                    # bias path: per i-row, [128,3,128] tile (p = j//3),
                    # transpose to [d, 384 j'] and project to b[j'-chunk, h]
                    bin4 = bt_p.tile([D, 4, 3, D], f32, tag="bin")
                    nc.sync.dma_start(
                        out=bin4.rearrange("p s t d -> p (s t d)"),
                        in_=bass.AP(
                            tensor=bias_c.ap().tensor,
                            offset=4 * g * L * D,
                            ap=[[3 * D, D], [L * D, 4], [1, 3 * D]],
                        ),
                    )
                    for s4 in range(4):
                        i_row = 4 * g + s4
                        bin_t = bin4[:, s4]
                        btp = btp_p.tile([D, L], f32, tag="btp")
                        for t in range(3):
                            nc.tensor.transpose(
                                out=btp[:, t * D : (t + 1) * D],
                                in_=bin_t[:, t, :],
                                identity=id_f32,
                            )
                        bts = bt_p.tile([D, L], f32, tag="bts")
                        nc.scalar.activation(out=bts, in_=btp, func=Identity)
                        bpp = bps_p.tile([D, 3, H], f32, tag="bps")
                        for t in range(3):
                            nc.tensor.matmul(
                                bpp[:, t, :],
                                bts[:, t * D : (t + 1) * D],
                                wb_sb,
                                start=True,
                                stop=True,
                            )
                        nc.vector.tensor_copy(
                            out=b_st[:, :, :, i_row], in_=bpp
                        )


                for ic in range(3):
                    lsb = ls_p.tile([D, H, L], bf16, tag="lsb")
                    for h in range(H):
                        pl = lg_p.tile([D, L], f32, tag="lg")
                        for g in range(NG):
                            nc.tensor.matmul(
                                pl,
                                qsh[:, h, g, ic * D : (ic + 1) * D],
                                ksh[:, h, g, :],
                                start=(g == 0),
                                stop=(g == NG - 1),
                            )
                        nc.scalar.activation(
                            out=lsb[:, h, :], in_=pl, func=Identity
                        )
                    ldst = bass.AP(
                        tensor=logits_dram.ap().tensor,
                        offset=ic * H * L,
                        ap=[[3 * H * L, D], [1, H * L]],
                    )
                    nc.sync.dma_start(
                        out=ldst, in_=lsb.rearrange("p h l -> p (h l)")
                    )
            qk_ctx.close()


            # ---------------- collective 1: ReduceScatter over i ----------------
            nc.gpsimd.collective_compute(
                "ReduceScatter",
                add,
                replica_groups=groups,
                ins=[logits_dram.ap().opt()],
                outs=[rs_out.ap().opt()],
            )

            # ---------------- gate path second half (overlaps ReduceScatter) ----
            with ExitStack() as ph2:
                gate_rows(ph2, "b_", 24, R)

            # ---------------- softmax on the i-shard ----------------
            with ExitStack() as ph3:
                sm_p = ph3.enter_context(tc.tile_pool(name="sm", bufs=1))
                smp_p = ph3.enter_context(
                    tc.tile_pool(name="smp", bufs=2, space="PSUM")
                )
                rs_sb = sm_p.tile([R, H, L], bf16)
                b2_sb = sm_p.tile([R, H, 3, D], f32)
                nc.sync.dma_start(
                    out=rs_sb.rearrange("i h l -> i (h l)"),
                    in_=rs_out.ap().rearrange("i h l -> i (h l)"),
                )
                for h in range(H):
                    for jc in range(3):
                        btp2 = smp_p.tile([R, D], f32, tag="btp2")
                        nc.tensor.transpose(
                            out=btp2, in_=b_st[:, jc, h, :], identity=id_f32
                        )
                        nc.vector.tensor_copy(out=b2_sb[:, h, jc, :], in_=btp2)
                ex_in = sm_p.tile([R, H, L], f32)
                nc.vector.tensor_add(
                    out=ex_in,
                    in0=rs_sb,
                    in1=b2_sb.rearrange("i h t d -> i h (t d)"),
                )
                exp_sb = sm_p.tile([R, H, L], f32)
                sums = sm_p.tile([R, H], f32)
                for h in range(H):
                    nc.scalar.activation(
                        out=exp_sb[:, h, :], in_=ex_in[:, h, :], func=Exp,
                        accum_out=sums[:, h : h + 1],
                    )
                rsum = sm_p.tile([R, H], f32)
                nc.vector.reciprocal(out=rsum, in_=sums)
                attn_sb = sm_p.tile([R, H, L], bf16)
                for h in range(H):
                    nc.gpsimd.tensor_scalar(
                        out=attn_sb[:, h, :],
                        in0=exp_sb[:, h, :],
                        scalar1=rsum[:, h : h + 1],
                        scalar2=None,
                        op0=mult,
                    )
                nc.sync.dma_start(out=attn_bounce[:, :, :], in_=attn_sb)

            # ---------------- collective 2: AllGather attn ----------------
            nc.gpsimd.collective_compute(
                "AllGather",
                mybir.AluOpType.bypass,
                replica_groups=groups,
                ins=[attn_bounce.ap().opt()],
                outs=[attn_full.ap().opt()],
            )

            # ---------------- phase 3: attn^T, AV, gate, Wo ----------------
            tc.strict_bb_all_engine_barrier()
            with ExitStack() as ph4:
                at_in_p = ph4.enter_context(tc.tile_pool(name="atin", bufs=3))
                at_ps_p = ph4.enter_context(
                    tc.tile_pool(name="atps", bufs=2, space="PSUM")
                )
                at_st = ph4.enter_context(tc.tile_pool(name="atst", bufs=1))
                attnT = at_st.tile([D, H, 3, L], bf16)  # [j, h, jc, x]
                at_in = [at_in_p.tile([D, H, L], bf16, tag="atin", name=f"at_in_{i}") for i in range(3)]
                for ic in range(3):
                    asrc = bass.AP(
                        tensor=attn_full.ap().tensor,
                        offset=ic * H * L,
                        ap=[[3 * H * L, D], [1, H * L]],
                    )
                    nc.sync.dma_start(
                        out=at_in[ic].rearrange("p h l -> p (h l)"), in_=asrc
                    )
                for h in range(H):
                    for jc in range(3):
                        pt = at_ps_p.tile([D, L], bf16, tag="atps")
                        for ic in range(3):
                            nc.tensor.transpose(
                                out=pt[:, ic * D : (ic + 1) * D],
                                in_=at_in[ic][:, h, jc * D : (jc + 1) * D],
                                identity=id_bf,
                            )
                        nc.scalar.activation(
                            out=attnT[:, h, jc, :], in_=pt, func=Identity
                        )

                av_p = ph4.enter_context(
                    tc.tile_pool(name="av", bufs=3, space="PSUM")
                )
                gt_p = ph4.enter_context(tc.tile_pool(name="gt", bufs=4))
                wo_ps = ph4.enter_context(
                    tc.tile_pool(name="wops", bufs=3, space="PSUM")
                )
                os_p = ph4.enter_context(tc.tile_pool(name="osb", bufs=4))
                for y in range(R):
                    pav = av_p.tile([D, L], f32, tag="av")
                    for h in range(H):
                        for jc in range(3):
                            nc.tensor.matmul(
                                pav[h * DH : (h + 1) * DH, :],
                                v_st[:, jc, y, h * DH : (h + 1) * DH],
                                attnT[:, h, jc, :],
                                start=(jc == 0),
                                stop=(jc == 2),
                                tile_position=(0, h * DH),
                            )
                    gated = gt_p.tile([D, L], bf16, tag="gt")
                    nc.vector.scalar_tensor_tensor(
                        out=gated,
                        in0=pav,
                        scalar=cv_sb,
                        in1=g_st[:, y, :],
                        op0=add,
                        op1=mult,
                    )
                    if y % 4 == 0:
                        osb4 = os_p.tile([D, 4, 3, D], f32, tag="osb",
                                         name=f"osb4_{y}")
                    for xc in range(3):
                        pwo = wo_ps.tile([D, D], f32, tag="wops")
                        nc.tensor.matmul(
                            pwo,
                            gated[:, xc * D : (xc + 1) * D],
                            wo_sb,
                            start=True,
                            stop=True,
                        )
                        nc.vector.tensor_add(
                            out=osb4[:, y % 4, xc, :], in0=pwo, in1=bo_bc
                        )
                    if y % 4 == 3:
                        nc.sync.dma_start(
                            out=bass.AP(
                                tensor=out_p.ap().tensor,
                                offset=(y - 3) * L * D,
                                ap=[[3 * D, D], [L * D, 4], [1, 3 * D]],
                            ),
                            in_=osb4.rearrange("p s t d -> p (s t d)"),
                        )

    nc.compile()
    return nc


def _prep_host(inputs):
    """Host-side: shard inputs, fold LN scale/bias + constants into weights."""
    f32 = np.float32
    bf = ml_dtypes.bfloat16
    pair = np.ascontiguousarray(np.asarray(inputs["pair"], f32)[0])
    bias = np.ascontiguousarray(np.asarray(inputs["bias"], f32)[0])
    ln_scale = np.asarray(inputs["ln_scale"], f32)
    ln_bias = np.asarray(inputs["ln_bias"], f32)
    Wq = np.asarray(inputs["Wq"], f32)
    Wk = np.asarray(inputs["Wk"], f32)
    Wv = np.asarray(inputs["Wv"], f32)
    Wb = np.asarray(inputs["Wb"], f32)
    Wg = np.asarray(inputs["Wg"], f32)
    bg = np.asarray(inputs["bg"], f32)
    Wo = np.asarray(inputs["Wo"], f32)
    bo = np.asarray(inputs["bo"], f32)

    Wq_eff = Wq * ln_scale[None, :] * SCALING
    Wk_eff = Wk * ln_scale[None, :] * KSCALE
    cq = (Wq @ ln_bias) * SCALING
    ck = (Wk @ ln_bias) * KSCALE

    def scat(W_eff):
        w = np.zeros((16, D, D), f32)
        for h in range(H):
            for s in range(4):
                for kk in range(DH):
                    w[h * 4 + s, :, kk * 4 + s] = W_eff[h * DH + kk, :]
        return w.astype(bf)

    wq_scat = scat(Wq_eff)
    wk_scat = scat(Wk_eff)
    cq_sh = np.zeros((H, D), f32)
    ck_sh = np.zeros((H, D), f32)
    for h in range(H):
        for s in range(4):
            for kk in range(DH):
                cq_sh[h, kk * 4 + s] = cq[h * DH + kk]
                ck_sh[h, kk * 4 + s] = ck[h * DH + kk]

    shared = {
        "wq_scat": wq_scat,
        "wk_scat": wk_scat,
        "wv_t": (Wv * ln_scale[None, :]).T.astype(bf).copy(),
        "wg_t": (Wg * ln_scale[None, :]).T.astype(bf).copy(),
        "wo_t": Wo.T.astype(bf).copy(),
        "wb_t": Wb.T.astype(f32).copy(),
        "cq_sh": cq_sh,
        "ck_sh": ck_sh,
        "cg_v": (Wg @ ln_bias + bg).astype(f32),
        "cv_v": (Wv @ ln_bias).astype(f32),
        "bo_v": bo.astype(f32),
    }
    in_maps = []
    for c in range(NCORES):
        sl = slice(c * R, (c + 1) * R)
        m = dict(shared)
        m["x_qkv"] = np.ascontiguousarray(pair[:, sl, :].transpose(1, 0, 2))
        m["x_gate"] = np.ascontiguousarray(pair[sl, :, :])
        m["bias_c"] = np.ascontiguousarray(bias[sl, :, :])
        in_maps.append(m)
    return in_maps


def kernel(**inputs):
    import os
    from concourse.bass_utils import run_bass_kernel_spmd

    in_maps = _prep_host(inputs)
    if "nc" not in _CACHE:
        _CACHE["nc"] = _build_graph()
    nc = _CACHE["nc"]
    kw = {}
    if os.environ.get("BAX_TRACE"):
        kw = dict(trace=True, tmpdir=os.environ.get("BAX_TRACE_DIR") or None)
    res = run_bass_kernel_spmd(nc, in_maps, list(range(NCORES)), **kw)
    _CACHE["last_result"] = res
    out = np.zeros((1, L, L, D), np.float32)
    for c in range(NCORES):
        out[0, c * R : (c + 1) * R, :, :] = res.results[c]["out"]
    return out


if __name__ == "__main__":
    nc = _build_graph()
    print("graph built ok")



# revision 4
# speedup vs baseline: 1.0230x; 1.0230x over previous
"""BiasedAxialAttention (row-attention path) distributed over 8 TRN2 NeuronCores.

Sharding: outer (non-attended) L axis "n" (= p axis 1 after the reference's
permute, = pair axis 2) split into 8 slices of 48 rows.

Per-core dataflow (all shapes hardcoded for B=1, L=384, D=128, H=4, DH=32):
  phase 1: LN(x_qkv) -> q,k projections emitted directly in the shuffled
           [(k*4+s), i] layout via scatter-column weights (4 accumulating
           matmuls per 4-n group), v projection in [j, (h,d)] layout,
           b = bias @ Wb^T via PE-transposed bias tiles,
           logits[i,j,h] accumulated over 12 groups at K=128.
  phase 2: ReduceScatter(logits over i) -> +b -> softmax(j) -> AllGather.
           gate path (LN + Wg + sigmoid) overlaps the collectives.
  phase 3: transpose attn -> AV (per-head 32-row strips of one PSUM tile),
           gate multiply (+folded cv via softmax-sums-to-1), Wo, +bo, DMA out.

v1 perf changes vs the original baseline:
  - big inputs (x_qkv / x_gate / bias_c) fed as bf16 (half the HBM traffic)
  - zero GpSimd elementwise work (was ~717us busy at ~2us/op): LN normalize
    on DVE (tensor_scalar) for the QKV path and on ACT (activation with
    per-partition scale/bias) for the gate path; softmax scaling on DVE
  - LN stats: one multi-group bn_stats per row + batched sqrt/reciprocal
    per 4-row group (was 6 DVE ops + sqrt + recip per row)
  - gate path moved entirely under the collectives
"""

import math

import numpy as np
import ml_dtypes

H, DH, D, L = 4, 32, 128, 384
NCORES = 8
R = L // NCORES  # 48
SCALING = 1.0 / math.sqrt(DH)
KSCALE = 1.0 / math.sqrt(L)
EPS = 1e-5
NG = R // 4  # 12 groups of 4 n-rows

_CACHE = {}


def _build_graph():
    import concourse.bass as bass
    import concourse.tile as tile
    from concourse import bacc, mybir

    f32 = mybir.dt.float32
    bf16 = mybir.dt.bfloat16
    Exp = mybir.ActivationFunctionType.Exp
    Identity = mybir.ActivationFunctionType.Identity
    Sigmoid = mybir.ActivationFunctionType.Sigmoid
    Sqrt = mybir.ActivationFunctionType.Sqrt
    sub = mybir.AluOpType.subtract
    mult = mybir.AluOpType.mult
    add = mybir.AluOpType.add

    nc = bacc.Bacc(
        "TRN2", target_bir_lowering=False, debug=False, num_devices=NCORES
    )

    # ---- external params (per-core shards + shared prepped weights) ----
    x_qkv = nc.declare_dram_parameter("x_qkv", [R, L, D], bf16, isOutput=False)
    x_gate = nc.declare_dram_parameter("x_gate", [R, L, D], bf16, isOutput=False)
    bias_c = nc.declare_dram_parameter("bias_c", [R, L, D], bf16, isOutput=False)
    wq_scat = nc.declare_dram_parameter("wq_scat", [16, D, D], bf16, isOutput=False)
    wk_scat = nc.declare_dram_parameter("wk_scat", [16, D, D], bf16, isOutput=False)
    wv_t = nc.declare_dram_parameter("wv_t", [D, D], bf16, isOutput=False)
    wg_t = nc.declare_dram_parameter("wg_t", [D, D], bf16, isOutput=False)
    wo_t = nc.declare_dram_parameter("wo_t", [D, D], bf16, isOutput=False)
    wb_t = nc.declare_dram_parameter("wb_t", [D, H], bf16, isOutput=False)
    cq_sh = nc.declare_dram_parameter("cq_sh", [H, D], f32, isOutput=False)
    ck_sh = nc.declare_dram_parameter("ck_sh", [H, D], f32, isOutput=False)
    cg_v = nc.declare_dram_parameter("cg_v", [D], f32, isOutput=False)
    cv_v = nc.declare_dram_parameter("cv_v", [D], f32, isOutput=False)
    bo_v = nc.declare_dram_parameter("bo_v", [D], f32, isOutput=False)
    out_p = nc.declare_dram_parameter("out", [R, L, D], f32, isOutput=True)

    # ---- internal DRAM (collective bounces; outs must be Shared) ----
    logits_dram = nc.dram_tensor("logits_dram", [L, H, L], bf16)
    rs_out = nc.dram_tensor("rs_out", [R, H, L], bf16)
    attn_bounce = nc.dram_tensor("attn_bounce", [R, H, L], bf16)
    attn_full = nc.dram_tensor("attn_full", [L, H, L], bf16, addr_space="Shared")
    groups = [list(range(NCORES))]

    with tile.TileContext(nc) as tc:
        from contextlib import ExitStack

        with ExitStack() as top:
            consts = top.enter_context(tc.tile_pool(name="consts", bufs=1))

            # constant tiles
            id_bf = consts.tile([D, D], bf16)
            wqs_sb = consts.tile([D, 16, D], bf16)   # [d, (h,s), P]
            wks_sb = consts.tile([D, 16, D], bf16)
            wv_sb = consts.tile([D, D], bf16)
            wg_sb = consts.tile([D, D], bf16)
            wo_sb = consts.tile([D, D], bf16)
            wb_sb = consts.tile([D, H], bf16)
            cq_sb = consts.tile([D, H], f32)         # per-partition bias, col h
            ck_sb = consts.tile([D, H], f32)
            cg_sb = consts.tile([D, 1], f32)
            cv_sb = consts.tile([D, 1], f32)
            bo_bc = consts.tile([D, D], f32)         # bo broadcast along partitions
            eps_sb = consts.tile([D, 1], f32)

            from concourse.masks import make_identity

            make_identity(nc, id_bf)
            nc.sync.dma_start(out=wqs_sb, in_=wq_scat.ap().rearrange("s d p -> d s p"))
            nc.sync.dma_start(out=wks_sb, in_=wk_scat.ap().rearrange("s d p -> d s p"))
            nc.sync.dma_start(out=wv_sb, in_=wv_t[:, :])
            nc.sync.dma_start(out=wg_sb, in_=wg_t[:, :])
            nc.sync.dma_start(out=wo_sb, in_=wo_t[:, :])
            nc.sync.dma_start(out=wb_sb, in_=wb_t[:, :])
            nc.sync.dma_start(out=cq_sb, in_=cq_sh.ap().rearrange("h d -> d h"))
            nc.sync.dma_start(out=ck_sb, in_=ck_sh.ap().rearrange("h d -> d h"))
            nc.sync.dma_start(out=cg_sb, in_=cg_v.ap().unsqueeze(1))
            nc.sync.dma_start(out=cv_sb, in_=cv_v.ap().unsqueeze(1))
            nc.sync.dma_start(
                out=bo_bc, in_=bo_v.ap().unsqueeze(0).broadcast_to((D, D))
            )
            nc.vector.memset(eps_sb, EPS)

            # persistent stores
            stores = top.enter_context(tc.tile_pool(name="stores", bufs=1))
            v_st = stores.tile([D, 3, R, D], bf16)      # [j, jc, y, (h,d)]
            g_st = stores.tile([D, R, L], bf16)         # [(h,d), y, x]
            b_st = stores.tile([D, 3, H, R], bf16)       # [j-part, jc, h, i]

            def group_ln_stats(st_p, xin4, name):
                """Stats for 4 rows x 3 chunks: returns (mv, istd) [D,4,3,*]."""
                stt = st_p.tile([D, 4, 3, 6], f32, tag="st", name=name + "st")
                mv = st_p.tile([D, 4, 3, 2], f32, tag="mv", name=name + "mv")
                sd = st_p.tile([D, 4, 3], f32, tag="sd", name=name + "sd")
                istd = st_p.tile([D, 4, 3], f32, tag="istd", name=name + "istd")
                for s in range(4):
                    for t in range(3):
                        nc.vector.bn_stats(out=stt[:, s, t, :], in_=xin4[:, s, t, :])
                        nc.vector.bn_aggr(out=mv[:, s, t, :], in_=stt[:, s, t, :])
                nc.scalar.activation(
                    out=sd, in_=mv[:, :, :, 1], func=Sqrt, bias=eps_sb, scale=1.0
                )
                nc.vector.reciprocal(out=istd, in_=sd)
                return mv, istd

            # ---------------- phase 1: QKV + b + logits ----------------
            qk_ctx = ExitStack()
            qk_st = qk_ctx.enter_context(tc.tile_pool(name="qk_st", bufs=1))
            qsh = qk_st.tile([D, H, NG, L], bf16)   # [(k,s), h, g, i]
            ksh = qk_st.tile([D, H, NG, L], bf16)
            with ExitStack() as ph1:
                xin_p = ph1.enter_context(tc.tile_pool(name="xin", bufs=4))
                st_p = ph1.enter_context(tc.tile_pool(name="stats", bufs=4))
                xh_p = ph1.enter_context(tc.tile_pool(name="xh", bufs=4))
                xt_p = ph1.enter_context(
                    tc.tile_pool(name="xt", bufs=2, space="PSUM")
                )
                xts_p = ph1.enter_context(tc.tile_pool(name="xts", bufs=6))
                vps_p = ph1.enter_context(
                    tc.tile_pool(name="vps", bufs=1, space="PSUM")
                )
                slab_p = ph1.enter_context(
                    tc.tile_pool(name="slab", bufs=5, space="PSUM")
                )

                def load_x4(src_dram, g):
                    xin4 = xin_p.tile([D, 4, 3, D], bf16, tag="xin")
                    nc.sync.dma_start(
                        out=xin4.rearrange("p s t d -> p (s t d)"),
                        in_=bass.AP(
                            tensor=src_dram.ap().tensor,
                            offset=4 * g * L * D,
                            ap=[[3 * D, D], [L * D, 4], [1, 3 * D]],
                        ),
                    )
                    return xin4

                for g in range(NG):
                    xts_g = []
                    xin4 = load_x4(x_qkv, g)
                    mv, istd = group_ln_stats(st_p, xin4, f"q{g}")
                    psq = [slab_p.tile([D, L], f32, tag="slab", name=f"psq_{g}_{h}") for h in range(H)]
                    for s in range(4):
                        n = 4 * g + s
                        xh = xh_p.tile([D, 3, D], bf16, tag="xh")
                        for t in range(3):
                            nc.vector.tensor_scalar(
                                out=xh[:, t, :],
                                in0=xin4[:, s, t, :],
                                scalar1=mv[:, s, t, 0:1],
                                scalar2=istd[:, s, t : t + 1],
                                op0=sub,
                                op1=mult,
                            )
                        xt = xt_p.tile([D, L], bf16, tag="xt")
                        for t in range(3):
                            nc.tensor.transpose(
                                out=xt[:, t * D : (t + 1) * D], in_=xh[:, t, :],
                                identity=id_bf,
                            )
                        xts = xts_p.tile([D, L], bf16, tag="xts")
                        if n % 2 == 0:
                            nc.vector.tensor_copy(out=xts, in_=xt)
                        else:
                            nc.scalar.activation(out=xts, in_=xt, func=Identity)
                        xts_g.append(xts)
                        # v projection: [j-chunk, (h,d)] x3 into one psum bank
                        vps = vps_p.tile([D, 3, D], f32, tag="vps")
                        for jc in range(3):
                            nc.tensor.matmul(
                                vps[:, jc, :],
                                xts[:, jc * D : (jc + 1) * D],
                                wv_sb,
                                start=True,
                                stop=True,
                            )
                        nc.vector.tensor_copy(out=v_st[:, :, n, :], in_=vps)
                        # q scattered projections accumulate into 4 head slabs
                        for h in range(H):
                            nc.tensor.matmul(
                                psq[h],
                                wqs_sb[:, h * 4 + s, :],
                                xts,
                                start=(s == 0),
                                stop=(s == 3),
                            )
                    for h in range(H):
                        nc.scalar.activation(
                            out=qsh[:, h, g, :], in_=psq[h], func=Identity,
                            bias=cq_sb[:, h : h + 1], scale=1.0,
                        )
                    psk = [slab_p.tile([D, L], f32, tag="slab", name=f"psk_{g}_{h}") for h in range(H)]
                    for s in range(4):
                        for h in range(H):
                            nc.tensor.matmul(
                                psk[h],
                                wks_sb[:, h * 4 + s, :],
                                xts_g[s],
                                start=(s == 0),
                                stop=(s == 3),
                            )
                    for h in range(H):
                        nc.scalar.activation(
                            out=ksh[:, h, g, :], in_=psk[h], func=Identity,
                            bias=ck_sb[:, h : h + 1], scale=1.0,
                        )

            # logits: [i-chunk, j] per head, K=128 over 12 groups
            with ExitStack() as phl:
                lg_p = phl.enter_context(
                    tc.tile_pool(name="lgp", bufs=2, space="PSUM")
                )
                ls_p = phl.enter_context(tc.tile_pool(name="lsb", bufs=3))
                bt_p = phl.enter_context(tc.tile_pool(name="bt", bufs=2))
                btp_p = phl.enter_context(
                    tc.tile_pool(name="btp", bufs=2, space="PSUM")
                )
                bps_p = phl.enter_context(
                    tc.tile_pool(name="bps", bufs=1, space="PSUM")
                )
                for g in range(NG):
                    # bias path: per i-row, [128,3,128] tile (p = j//3),
                    # transpose to [d, 384 j'] and project to b[j'-chunk, h]
                    bin4 = bt_p.tile([D, 4, 3, D], bf16, tag="bin")
                    nc.sync.dma_start(
                        out=bin4.rearrange("p s t d -> p (s t d)"),
                        in_=bass.AP(
                            tensor=bias_c.ap().tensor,
                            offset=4 * g * L * D,
                            ap=[[3 * D, D], [L * D, 4], [1, 3 * D]],
                        ),
                    )
                    for s4 in range(4):
                        i_row = 4 * g + s4
                        bin_t = bin4[:, s4]
                        btp = btp_p.tile([D, L], bf16, tag="btp")
                        for t in range(3):
                            nc.tensor.transpose(
                                out=btp[:, t * D : (t + 1) * D],
                                in_=bin_t[:, t, :],
                                identity=id_bf,
                            )
                        bts = bt_p.tile([D, L], bf16, tag="bts")
                        nc.scalar.activation(out=bts, in_=btp, func=Identity)
                        bpp = bps_p.tile([D, 3, H], f32, tag="bps")
                        for t in range(3):
                            nc.tensor.matmul(
                                bpp[:, t, :],
                                bts[:, t * D : (t + 1) * D],
                                wb_sb,
                                start=True,
                                stop=True,
                            )
                        nc.vector.tensor_copy(
                            out=b_st[:, :, :, i_row], in_=bpp
                        )

                for ic in range(3):
                    lsb = ls_p.tile([D, H, L], bf16, tag="lsb")
                    for h in range(H):
                        pl = lg_p.tile([D, L], f32, tag="lg")
                        for g in range(NG):
                            nc.tensor.matmul(
                                pl,
                                qsh[:, h, g, ic * D : (ic + 1) * D],
                                ksh[:, h, g, :],
                                start=(g == 0),
                                stop=(g == NG - 1),
                            )
                        nc.scalar.activation(
                            out=lsb[:, h, :], in_=pl, func=Identity
                        )
                    ldst = bass.AP(
                        tensor=logits_dram.ap().tensor,
                        offset=ic * H * L,
                        ap=[[3 * H * L, D], [1, H * L]],
                    )
                    nc.sync.dma_start(
                        out=ldst, in_=lsb.rearrange("p h l -> p (h l)")
                    )
            qk_ctx.close()

            # ---------------- collective 1: ReduceScatter over i ----------------
            nc.gpsimd.collective_compute(
                "ReduceScatter",
                add,
                replica_groups=groups,
                ins=[logits_dram.ap().opt()],
                outs=[rs_out.ap().opt()],
            )

            # ---------------- gate path (overlaps the collectives) ----------
            def gate_rows(scope, pfx, y0, y1):
                xin_p = scope.enter_context(
                    tc.tile_pool(name=pfx + "xin2", bufs=3)
                )
                st_p = scope.enter_context(
                    tc.tile_pool(name=pfx + "stats2", bufs=3)
                )
                xh_p = scope.enter_context(
                    tc.tile_pool(name=pfx + "xh2", bufs=4)
                )
                xt_p = scope.enter_context(
                    tc.tile_pool(name=pfx + "xt2", bufs=2, space="PSUM")
                )
                xts_p = scope.enter_context(
                    tc.tile_pool(name=pfx + "xts2", bufs=4)
                )
                gp_p = scope.enter_context(
                    tc.tile_pool(name=pfx + "gp", bufs=2, space="PSUM")
                )
                for g in range(y0 // 4, y1 // 4):
                    xin4g = xin_p.tile([D, 4, 3, D], bf16, tag="xin",
                                       name=f"{pfx}xin4g_{g}")
                    nc.sync.dma_start(
                        out=xin4g.rearrange("p s t d -> p (s t d)"),
                        in_=bass.AP(
                            tensor=x_gate.ap().tensor,
                            offset=4 * g * L * D,
                            ap=[[3 * D, D], [L * D, 4], [1, 3 * D]],
                        ),
                    )
                    mv, istd = group_ln_stats(st_p, xin4g, f"{pfx}{g}")
                    negmi = st_p.tile([D, 4, 3], f32, tag="negmi",
                                      name=f"{pfx}negmi{g}")
                    nc.vector.scalar_tensor_tensor(
                        out=negmi,
                        in0=mv[:, :, :, 0],
                        scalar=-1.0,
                        in1=istd,
                        op0=mult,
                        op1=mult,
                    )
                    for s in range(4):
                        y = 4 * g + s
                        xh = xh_p.tile([D, 3, D], bf16, tag="xh")
                        for t in range(3):
                            nc.scalar.activation(
                                out=xh[:, t, :],
                                in_=xin4g[:, s, t, :],
                                func=Identity,
                                bias=negmi[:, s, t : t + 1],
                                scale=istd[:, s, t : t + 1],
                            )
                        xt = xt_p.tile([D, L], bf16, tag="xt")
                        for t in range(3):
                            nc.tensor.transpose(
                                out=xt[:, t * D : (t + 1) * D], in_=xh[:, t, :],
                                identity=id_bf,
                            )
                        xts = xts_p.tile([D, L], bf16, tag="xts")
                        nc.vector.tensor_copy(out=xts, in_=xt)
                        gp = gp_p.tile([D, L], f32, tag="gp")
                        nc.tensor.matmul(gp, wg_sb, xts, start=True, stop=True)
                        nc.scalar.activation(
                            out=g_st[:, y, :], in_=gp, func=Sigmoid, bias=cg_sb,
                            scale=1.0,
                        )

            with ExitStack() as ph2:
                gate_rows(ph2, "a_", 0, 24)

            # ---------------- softmax on the i-shard ----------------
            with ExitStack() as ph3:
                sm_p = ph3.enter_context(tc.tile_pool(name="sm", bufs=1))
                smp_p = ph3.enter_context(
                    tc.tile_pool(name="smp", bufs=2, space="PSUM")
                )
                rs_sb = sm_p.tile([R, H, L], bf16)
                b2_sb = sm_p.tile([R, H, 3, D], bf16)
                nc.sync.dma_start(
                    out=rs_sb.rearrange("i h l -> i (h l)"),
                    in_=rs_out.ap().rearrange("i h l -> i (h l)"),
                )
                for h in range(H):
                    for jc in range(3):
                        btp2 = smp_p.tile([R, D], bf16, tag="btp2")
                        nc.tensor.transpose(
                            out=btp2, in_=b_st[:, jc, h, :], identity=id_bf
                        )
                        nc.vector.tensor_copy(out=b2_sb[:, h, jc, :], in_=btp2)
                ex_in = sm_p.tile([R, H, L], f32)
                nc.vector.tensor_add(
                    out=ex_in,
                    in0=rs_sb,
                    in1=b2_sb.rearrange("i h t d -> i h (t d)"),
                )
                exp_sb = sm_p.tile([R, H, L], f32)
                sums = sm_p.tile([R, H], f32)
                for h in range(H):
                    nc.scalar.activation(
                        out=exp_sb[:, h, :], in_=ex_in[:, h, :], func=Exp,
                        accum_out=sums[:, h : h + 1],
                    )
                rsum = sm_p.tile([R, H], f32)
                nc.vector.reciprocal(out=rsum, in_=sums)
                attn_sb = sm_p.tile([R, H, L], bf16)
                for h in range(H):
                    nc.vector.tensor_scalar(
                        out=attn_sb[:, h, :],
                        in0=exp_sb[:, h, :],
                        scalar1=rsum[:, h : h + 1],
                        scalar2=None,
                        op0=mult,
                    )
                nc.sync.dma_start(out=attn_bounce[:, :, :], in_=attn_sb)

            # ---------------- collective 2: AllGather attn ----------------
            nc.gpsimd.collective_compute(
                "AllGather",
                mybir.AluOpType.bypass,
                replica_groups=groups,
                ins=[attn_bounce.ap().opt()],
                outs=[attn_full.ap().opt()],
            )

            # ---------------- gate second half (overlaps AllGather) --------
            with ExitStack() as ph2b:
                gate_rows(ph2b, "b_", 24, R)

            # ---------------- phase 3: attn^T, AV, gate, Wo ----------------
            tc.strict_bb_all_engine_barrier()
            with ExitStack() as ph4:
                at_in_p = ph4.enter_context(tc.tile_pool(name="atin", bufs=3))
                at_ps_p = ph4.enter_context(
                    tc.tile_pool(name="atps", bufs=2, space="PSUM")
                )
                at_st = ph4.enter_context(tc.tile_pool(name="atst", bufs=1))
                attnT = at_st.tile([D, H, 3, L], bf16)  # [j, h, jc, x]
                at_in = [at_in_p.tile([D, H, L], bf16, tag="atin", name=f"at_in_{i}") for i in range(3)]
                for ic in range(3):
                    asrc = bass.AP(
                        tensor=attn_full.ap().tensor,
                        offset=ic * H * L,
                        ap=[[3 * H * L, D], [1, H * L]],
                    )
                    nc.sync.dma_start(
                        out=at_in[ic].rearrange("p h l -> p (h l)"), in_=asrc
                    )
                for h in range(H):
                    for jc in range(3):
                        pt = at_ps_p.tile([D, L], bf16, tag="atps")
                        for ic in range(3):
                            nc.tensor.transpose(
                                out=pt[:, ic * D : (ic + 1) * D],
                                in_=at_in[ic][:, h, jc * D : (jc + 1) * D],
                                identity=id_bf,
                            )
                        nc.scalar.activation(
                            out=attnT[:, h, jc, :], in_=pt, func=Identity
                        )

                av_p = ph4.enter_context(
                    tc.tile_pool(name="av", bufs=3, space="PSUM")
                )
                gt_p = ph4.enter_context(tc.tile_pool(name="gt", bufs=4))
                wo_ps = ph4.enter_context(
                    tc.tile_pool(name="wops", bufs=3, space="PSUM")
                )
                os_p = ph4.enter_context(tc.tile_pool(name="osb", bufs=4))
                for y in range(R):
                    pav = av_p.tile([D, L], f32, tag="av")
                    for h in range(H):
                        for jc in range(3):
                            nc.tensor.matmul(
                                pav[h * DH : (h + 1) * DH, :],
                                v_st[:, jc, y, h * DH : (h + 1) * DH],
                                attnT[:, h, jc, :],
                                start=(jc == 0),
                                stop=(jc == 2),
                                tile_position=(0, h * DH),
                            )
                    gated = gt_p.tile([D, L], bf16, tag="gt")
                    nc.vector.scalar_tensor_tensor(
                        out=gated,
                        in0=pav,
                        scalar=cv_sb,
                        in1=g_st[:, y, :],
                        op0=add,
                        op1=mult,
                    )
                    if y % 4 == 0:
                        osb4 = os_p.tile([D, 4, 3, D], f32, tag="osb",
                                         name=f"osb4_{y}")
                    for xc in range(3):
                        pwo = wo_ps.tile([D, D], f32, tag="wops")
                        nc.tensor.matmul(
                            pwo,
                            gated[:, xc * D : (xc + 1) * D],
                            wo_sb,
                            start=True,
                            stop=True,
                        )
                        nc.vector.tensor_add(
                            out=osb4[:, y % 4, xc, :], in0=pwo, in1=bo_bc
                        )
                    if y % 4 == 3:
                        nc.sync.dma_start(
                            out=bass.AP(
                                tensor=out_p.ap().tensor,
                                offset=(y - 3) * L * D,
                                ap=[[3 * D, D], [L * D, 4], [1, 3 * D]],
                            ),
                            in_=osb4.rearrange("p s t d -> p (s t d)"),
                        )

    nc.compile()
    return nc


def _prep_host(inputs):
    """Host-side: shard inputs, fold LN scale/bias + constants into weights."""
    f32 = np.float32
    bf = ml_dtypes.bfloat16
    pair = np.ascontiguousarray(np.asarray(inputs["pair"], f32)[0])
    bias = np.ascontiguousarray(np.asarray(inputs["bias"], f32)[0])
    ln_scale = np.asarray(inputs["ln_scale"], f32)
    ln_bias = np.asarray(inputs["ln_bias"], f32)
    Wq = np.asarray(inputs["Wq"], f32)
    Wk = np.asarray(inputs["Wk"], f32)
    Wv = np.asarray(inputs["Wv"], f32)
    Wb = np.asarray(inputs["Wb"], f32)
    Wg = np.asarray(inputs["Wg"], f32)
    bg = np.asarray(inputs["bg"], f32)
    Wo = np.asarray(inputs["Wo"], f32)
    bo = np.asarray(inputs["bo"], f32)

    Wq_eff = Wq * ln_scale[None, :] * SCALING
    Wk_eff = Wk * ln_scale[None, :] * KSCALE
    cq = (Wq @ ln_bias) * SCALING
    ck = (Wk @ ln_bias) * KSCALE

    def scat(W_eff):
        w = np.zeros((16, D, D), f32)
        for h in range(H):
            for s in range(4):
                for kk in range(DH):
                    w[h * 4 + s, :, kk * 4 + s] = W_eff[h * DH + kk, :]
        return w.astype(bf)

    wq_scat = scat(Wq_eff)
    wk_scat = scat(Wk_eff)
    cq_sh = np.zeros((H, D), f32)
    ck_sh = np.zeros((H, D), f32)
    for h in range(H):
        for s in range(4):
            for kk in range(DH):
                cq_sh[h, kk * 4 + s] = cq[h * DH + kk]
                ck_sh[h, kk * 4 + s] = ck[h * DH + kk]

    pair_bf = pair.astype(bf)
    bias_bf = bias.astype(bf)

    shared = {
        "wq_scat": wq_scat,
        "wk_scat": wk_scat,
        "wv_t": (Wv * ln_scale[None, :]).T.astype(bf).copy(),
        "wg_t": (Wg * ln_scale[None, :]).T.astype(bf).copy(),
        "wo_t": Wo.T.astype(bf).copy(),
        "wb_t": Wb.T.astype(bf).copy(),
        "cq_sh": cq_sh,
        "ck_sh": ck_sh,
        "cg_v": (Wg @ ln_bias + bg).astype(f32),
        "cv_v": (Wv @ ln_bias).astype(f32),
        "bo_v": bo.astype(f32),
    }
    in_maps = []
    for c in range(NCORES):
        sl = slice(c * R, (c + 1) * R)
        m = dict(shared)
        m["x_qkv"] = np.ascontiguousarray(pair_bf[:, sl, :].transpose(1, 0, 2))
        m["x_gate"] = np.ascontiguousarray(pair_bf[sl, :, :])
        m["bias_c"] = np.ascontiguousarray(bias_bf[sl, :, :])
        in_maps.append(m)
    return in_maps


def kernel(**inputs):
    import os
    from concourse.bass_utils import run_bass_kernel_spmd

    in_maps = _prep_host(inputs)
    if "nc" not in _CACHE:
        _CACHE["nc"] = _build_graph()
    nc = _CACHE["nc"]
    kw = {}
    if os.environ.get("BAX_TRACE"):
        kw = dict(trace=True, tmpdir=os.environ.get("BAX_TRACE_DIR") or None)
    res = run_bass_kernel_spmd(nc, in_maps, list(range(NCORES)), **kw)
    _CACHE["last_result"] = res
    out = np.zeros((1, L, L, D), np.float32)
    for c in range(NCORES):
        out[0, c * R : (c + 1) * R, :, :] = res.results[c]["out"]
    return out


if __name__ == "__main__":
    nc = _build_graph()
    print("graph built ok")


# revision 15
# speedup vs baseline: 1.0373x; 1.0140x over previous
"""BiasedAxialAttention (row-attention path) distributed over 8 TRN2 NeuronCores.

Sharding: outer (non-attended) L axis "n" (= p axis 1 after the reference's
permute, = pair axis 2) split into 8 slices of 48 rows.

Per-core dataflow (all shapes hardcoded for B=1, L=384, D=128, H=4, DH=32):
  phase 1: LN(x_qkv) -> q,k projections emitted directly in the shuffled
           [(k*4+s), i] layout via scatter-column weights (4 accumulating
           matmuls per 4-n group), v projection in [j, (h,d)] layout,
           b = bias @ Wb^T via PE-transposed bias tiles,
           logits[i,j,h] accumulated over 12 groups at K=128.
  phase 2: ReduceScatter(logits over i) -> +b -> softmax(j) -> AllGather.
           gate path (LN + Wg + sigmoid) overlaps the collectives.
  phase 3: transpose attn -> AV (per-head 32-row strips of one PSUM tile),
           gate multiply (+folded cv via softmax-sums-to-1), Wo, +bo, DMA out.

v1 perf changes vs the original baseline:
  - big inputs (x_qkv / x_gate / bias_c) fed as bf16 (half the HBM traffic)
  - zero GpSimd elementwise work (was ~717us busy at ~2us/op): LN normalize
    on DVE (tensor_scalar) for the QKV path and on ACT (activation with
    per-partition scale/bias) for the gate path; softmax scaling on DVE
  - LN stats: one multi-group bn_stats per row + batched sqrt/reciprocal
    per 4-row group (was 6 DVE ops + sqrt + recip per row)
  - gate path moved entirely under the collectives
"""

import math

import numpy as np
import ml_dtypes

H, DH, D, L = 4, 32, 128, 384
NCORES = 8
R = L // NCORES  # 48
SCALING = 1.0 / math.sqrt(DH)
KSCALE = 1.0 / math.sqrt(L)
EPS = 1e-5
NG = R // 4  # 12 groups of 4 n-rows

_CACHE = {}


def _build_graph():
    import concourse.bass as bass
    import concourse.tile as tile
    from concourse import bacc, mybir

    f32 = mybir.dt.float32
    bf16 = mybir.dt.bfloat16
    Exp = mybir.ActivationFunctionType.Exp
    Identity = mybir.ActivationFunctionType.Identity
    Sigmoid = mybir.ActivationFunctionType.Sigmoid
    Sqrt = mybir.ActivationFunctionType.Sqrt
    sub = mybir.AluOpType.subtract
    mult = mybir.AluOpType.mult
    add = mybir.AluOpType.add

    nc = bacc.Bacc(
        "TRN2", target_bir_lowering=False, debug=False, num_devices=NCORES
    )

    # ---- external params (per-core shards + shared prepped weights) ----
    x_qkv = nc.declare_dram_parameter("x_qkv", [R, L, D], bf16, isOutput=False)
    x_gate = nc.declare_dram_parameter("x_gate", [R, L, D], bf16, isOutput=False)
    bias_c = nc.declare_dram_parameter("bias_c", [R, L, D], bf16, isOutput=False)
    wq_t = nc.declare_dram_parameter("wq_t", [D, D], bf16, isOutput=False)
    wk_t = nc.declare_dram_parameter("wk_t", [D, D], bf16, isOutput=False)
    wv_t = nc.declare_dram_parameter("wv_t", [D, D], bf16, isOutput=False)
    wg_t = nc.declare_dram_parameter("wg_t", [D, D], bf16, isOutput=False)
    wo_t = nc.declare_dram_parameter("wo_t", [D, D], bf16, isOutput=False)
    wb_t = nc.declare_dram_parameter("wb_t", [D, H], bf16, isOutput=False)
    cq_sh = nc.declare_dram_parameter("cq_sh", [H, D], f32, isOutput=False)
    ck_sh = nc.declare_dram_parameter("ck_sh", [H, D], f32, isOutput=False)
    cg_v = nc.declare_dram_parameter("cg_v", [D], f32, isOutput=False)
    cv_v = nc.declare_dram_parameter("cv_v", [D], f32, isOutput=False)
    bo3_v = nc.declare_dram_parameter("bo3_v", [3 * D], bf16, isOutput=False)
    out_p = nc.declare_dram_parameter("out", [R, L, D], f32, isOutput=True)

    # ---- internal DRAM (collective bounces; outs must be Shared) ----
    logits_dram = nc.dram_tensor("logits_dram", [L, H, L], bf16)
    rs_out = nc.dram_tensor("rs_out", [R, H, L], bf16)
    attn_bounce = nc.dram_tensor("attn_bounce", [R, H, L], bf16)
    attn_full = nc.dram_tensor("attn_full", [L, H, L], bf16, addr_space="Shared")
    groups = [list(range(NCORES))]

    with tile.TileContext(nc) as tc:
        from contextlib import ExitStack

        with ExitStack() as top:
            consts = top.enter_context(tc.tile_pool(name="consts", bufs=1))

            # constant tiles
            id_bf = consts.tile([D, D], bf16)
            wqh_sb = consts.tile([D, H, DH], bf16)   # [d, h, dh]
            wkh_sb = consts.tile([D, H, DH], bf16)
            ones_row = consts.tile([1, D], bf16)     # K=1 stationary for bo
            bo3_row = consts.tile([1, 3 * D], bf16)  # bo tiled x3, K=1 moving
            wv_sb = consts.tile([D, D], bf16)
            wg_sb = consts.tile([D, D], bf16)
            wo_sb = consts.tile([D, D], bf16)
            wb_sb = consts.tile([D, H], bf16)
            cq_sb = consts.tile([D, H], f32)         # per-partition bias, col h
            ck_sb = consts.tile([D, H], f32)
            cg_sb = consts.tile([D, 1], f32)
            cv_sb = consts.tile([D, 1], f32)
            eps_sb = consts.tile([D, 1], f32)

            from concourse.masks import make_identity

            make_identity(nc, id_bf)
            nc.sync.dma_start(
                out=wqh_sb.rearrange("d h k -> d (h k)"), in_=wq_t[:, :]
            )
            nc.sync.dma_start(
                out=wkh_sb.rearrange("d h k -> d (h k)"), in_=wk_t[:, :]
            )
            nc.vector.memset(ones_row, 1.0)
            nc.sync.dma_start(out=bo3_row, in_=bo3_v.ap().unsqueeze(0))
            nc.sync.dma_start(out=wv_sb, in_=wv_t[:, :])
            nc.sync.dma_start(out=wg_sb, in_=wg_t[:, :])
            nc.sync.dma_start(out=wo_sb, in_=wo_t[:, :])
            nc.sync.dma_start(out=wb_sb, in_=wb_t[:, :])
            nc.sync.dma_start(out=cq_sb, in_=cq_sh.ap().rearrange("h d -> d h"))
            nc.sync.dma_start(out=ck_sb, in_=ck_sh.ap().rearrange("h d -> d h"))
            nc.sync.dma_start(out=cg_sb, in_=cg_v.ap().unsqueeze(1))
            nc.sync.dma_start(out=cv_sb, in_=cv_v.ap().unsqueeze(1))
            nc.vector.memset(eps_sb, EPS)

            # persistent stores
            stores = top.enter_context(tc.tile_pool(name="stores", bufs=1))
            v_st = stores.tile([D, 3, R, D], bf16)      # [j, jc, y, (h,d)]
            g_st = stores.tile([D, R, L], bf16)         # [(h,d), y, x]
            b_st = stores.tile([D, 3, H, R], bf16)       # [j-part, jc, h, i]

            def group_ln_stats(st_p, xin4, name):
                """Stats for 4 rows x 3 chunks: returns (mv, istd) [D,4,3,*]."""
                stt = st_p.tile([D, 4, 3, 6], f32, tag="st", name=name + "st")
                mv = st_p.tile([D, 4, 3, 2], f32, tag="mv", name=name + "mv")
                sd = st_p.tile([D, 4, 3], f32, tag="sd", name=name + "sd")
                istd = st_p.tile([D, 4, 3], f32, tag="istd", name=name + "istd")
                for s in range(4):
                    for t in range(3):
                        nc.vector.bn_stats(out=stt[:, s, t, :], in_=xin4[:, s, t, :])
                        nc.vector.bn_aggr(out=mv[:, s, t, :], in_=stt[:, s, t, :])
                nc.scalar.activation(
                    out=sd, in_=mv[:, :, :, 1], func=Sqrt, bias=eps_sb, scale=1.0
                )
                nc.vector.reciprocal(out=istd, in_=sd)
                return mv, istd

            # ---------------- phase 1: QKV + b + logits ----------------
            qk_ctx = ExitStack()
            qk_st = qk_ctx.enter_context(tc.tile_pool(name="qk_st", bufs=1))
            qsh = qk_st.tile([D, H, NG, L], bf16)   # [(k,s), h, g, i]
            ksh = qk_st.tile([D, H, NG, L], bf16)
            with ExitStack() as ph1:
                xin_p = ph1.enter_context(tc.tile_pool(name="xin", bufs=4))
                st_p = ph1.enter_context(tc.tile_pool(name="stats", bufs=4))
                xh_p = ph1.enter_context(tc.tile_pool(name="xh", bufs=4))
                xt_p = ph1.enter_context(
                    tc.tile_pool(name="xt", bufs=2, space="PSUM")
                )
                xts_p = ph1.enter_context(tc.tile_pool(name="xts", bufs=6))
                vps_p = ph1.enter_context(
                    tc.tile_pool(name="vps", bufs=1, space="PSUM")
                )
                slab_p = ph1.enter_context(
                    tc.tile_pool(name="slab", bufs=5, space="PSUM")
                )

                def load_x4(src_dram, g):
                    xin4 = xin_p.tile([D, 4, 3, D], bf16, tag="xin")
                    nc.sync.dma_start(
                        out=xin4.rearrange("p s t d -> p (s t d)"),
                        in_=bass.AP(
                            tensor=src_dram.ap().tensor,
                            offset=4 * g * L * D,
                            ap=[[3 * D, D], [L * D, 4], [1, 3 * D]],
                        ),
                    )
                    return xin4

                for g in range(NG):
                    xts_g = []
                    xin4 = load_x4(x_qkv, g)
                    mv, istd = group_ln_stats(st_p, xin4, f"q{g}")
                    psq = [slab_p.tile([D, L], f32, tag="slab", name=f"psq_{g}_{h}") for h in range(H)]
                    for s in range(4):
                        n = 4 * g + s
                        xh = xh_p.tile([D, 3, D], bf16, tag="xh")
                        for t in range(3):
                            nc.vector.tensor_scalar(
                                out=xh[:, t, :],
                                in0=xin4[:, s, t, :],
                                scalar1=mv[:, s, t, 0:1],
                                scalar2=istd[:, s, t : t + 1],
                                op0=sub,
                                op1=mult,
                            )
                        xt = xt_p.tile([D, L], bf16, tag="xt")
                        for t in range(3):
                            nc.tensor.transpose(
                                out=xt[:, t * D : (t + 1) * D], in_=xh[:, t, :],
                                identity=id_bf,
                            )
                        xts = xts_p.tile([D, L], bf16, tag="xts")
                        if n % 2 == 0:
                            nc.vector.tensor_copy(out=xts, in_=xt)
                        else:
                            nc.scalar.activation(out=xts, in_=xt, func=Identity)
                        xts_g.append(xts)
                        # v projection: [j-chunk, (h,d)] x3 into one psum bank
                        vps = vps_p.tile([D, 3, D], f32, tag="vps")
                        for jc in range(3):
                            nc.tensor.matmul(
                                vps[:, jc, :],
                                xts[:, jc * D : (jc + 1) * D],
                                wv_sb,
                                start=True,
                                stop=True,
                            )
                        nc.vector.tensor_copy(out=v_st[:, :, n, :], in_=vps)
                        # q projection strips: head h row-block s lands on
                        # psum partitions [32s, 32s+32) via col-tiling, so
                        # the 4 s-strips stream concurrently in the PE array
                        for h in range(H):
                            nc.tensor.matmul(
                                psq[h][DH * s : DH * (s + 1), :],
                                wqh_sb[:, h, :],
                                xts,
                                start=True,
                                stop=True,
                                tile_position=(0, DH * s),
                            )
                    for h in range(H):
                        nc.scalar.activation(
                            out=qsh[:, h, g, :], in_=psq[h], func=Identity,
                            bias=cq_sb[:, h : h + 1], scale=1.0,
                        )
                    psk = [slab_p.tile([D, L], f32, tag="slab", name=f"psk_{g}_{h}") for h in range(H)]
                    for s in range(4):
                        for h in range(H):
                            nc.tensor.matmul(
                                psk[h][DH * s : DH * (s + 1), :],
                                wkh_sb[:, h, :],
                                xts_g[s],
                                start=True,
                                stop=True,
                                tile_position=(0, DH * s),
                            )
                    for h in range(H):
                        nc.vector.tensor_scalar(
                            out=ksh[:, h, g, :],
                            in0=psk[h],
                            scalar1=ck_sb[:, h : h + 1],
                            scalar2=None,
                            op0=add,
                        )

            # logits: [i-chunk, j] per head, K=128 over 12 groups
            with ExitStack() as phl:
                lg_p = phl.enter_context(
                    tc.tile_pool(name="lgp", bufs=2, space="PSUM")
                )
                ls_p = phl.enter_context(tc.tile_pool(name="lsb", bufs=3))
                for ic in range(3):
                    lsb = ls_p.tile([D, H, L], bf16, tag="lsb")
                    for h in range(H):
                        pl = lg_p.tile([D, L], f32, tag="lg")
                        for g in range(NG):
                            nc.tensor.matmul(
                                pl,
                                qsh[:, h, g, ic * D : (ic + 1) * D],
                                ksh[:, h, g, :],
                                start=(g == 0),
                                stop=(g == NG - 1),
                            )
                        nc.scalar.activation(
                            out=lsb[:, h, :], in_=pl, func=Identity
                        )
                    ldst = bass.AP(
                        tensor=logits_dram.ap().tensor,
                        offset=ic * H * L,
                        ap=[[3 * H * L, D], [1, H * L]],
                    )
                    nc.sync.dma_start(
                        out=ldst, in_=lsb.rearrange("p h l -> p (h l)")
                    )
            qk_ctx.close()

            # ---------------- collective 1: ReduceScatter over i ----------------
            nc.gpsimd.collective_compute(
                "ReduceScatter",
                add,
                replica_groups=groups,
                ins=[logits_dram.ap().opt()],
                outs=[rs_out.ap().opt()],
            )

            # ---------------- gate path (overlaps the collectives) ----------
            def gate_rows(scope, pfx, y0, y1):
                xin_p = scope.enter_context(
                    tc.tile_pool(name=pfx + "xin2", bufs=3)
                )
                st_p = scope.enter_context(
                    tc.tile_pool(name=pfx + "stats2", bufs=3)
                )
                xh_p = scope.enter_context(
                    tc.tile_pool(name=pfx + "xh2", bufs=4)
                )
                xt_p = scope.enter_context(
                    tc.tile_pool(name=pfx + "xt2", bufs=2, space="PSUM")
                )
                xts_p = scope.enter_context(
                    tc.tile_pool(name=pfx + "xts2", bufs=4)
                )
                gp_p = scope.enter_context(
                    tc.tile_pool(name=pfx + "gp", bufs=2, space="PSUM")
                )
                for g in range(y0 // 4, y1 // 4):
                    xin4g = xin_p.tile([D, 4, 3, D], bf16, tag="xin",
                                       name=f"{pfx}xin4g_{g}")
                    nc.sync.dma_start(
                        out=xin4g.rearrange("p s t d -> p (s t d)"),
                        in_=bass.AP(
                            tensor=x_gate.ap().tensor,
                            offset=4 * g * L * D,
                            ap=[[3 * D, D], [L * D, 4], [1, 3 * D]],
                        ),
                    )
                    mv, istd = group_ln_stats(st_p, xin4g, f"{pfx}{g}")
                    negmi = st_p.tile([D, 4, 3], f32, tag="negmi",
                                      name=f"{pfx}negmi{g}")
                    nc.vector.scalar_tensor_tensor(
                        out=negmi,
                        in0=mv[:, :, :, 0],
                        scalar=-1.0,
                        in1=istd,
                        op0=mult,
                        op1=mult,
                    )
                    for s in range(4):
                        y = 4 * g + s
                        xh = xh_p.tile([D, 3, D], bf16, tag="xh")
                        for t in range(3):
                            nc.scalar.activation(
                                out=xh[:, t, :],
                                in_=xin4g[:, s, t, :],
                                func=Identity,
                                bias=negmi[:, s, t : t + 1],
                                scale=istd[:, s, t : t + 1],
                            )
                        xt = xt_p.tile([D, L], bf16, tag="xt")
                        for t in range(3):
                            nc.tensor.transpose(
                                out=xt[:, t * D : (t + 1) * D], in_=xh[:, t, :],
                                identity=id_bf,
                            )
                        xts = xts_p.tile([D, L], bf16, tag="xts")
                        nc.vector.tensor_copy(out=xts, in_=xt)
                        gp = gp_p.tile([D, L], f32, tag="gp")
                        nc.tensor.matmul(gp, wg_sb, xts, start=True, stop=True)
                        nc.scalar.activation(
                            out=g_st[:, y, :], in_=gp, func=Sigmoid, bias=cg_sb,
                            scale=1.0,
                        )

            # ---- bias path + gate first half: overlap the ReduceScatter ----
            with ExitStack() as ph2:
                bt_p = ph2.enter_context(tc.tile_pool(name="bt", bufs=2))
                btp_p = ph2.enter_context(
                    tc.tile_pool(name="btp", bufs=2, space="PSUM")
                )
                bps_p = ph2.enter_context(
                    tc.tile_pool(name="bps", bufs=1, space="PSUM")
                )
                for g in range(NG):
                    # bias path: per i-row, [128,3,128] tile (p = j//3),
                    # transpose to [d, 384 j'] and project to b[j'-chunk, h]
                    bin4 = bt_p.tile([D, 4, 3, D], bf16, tag="bin")
                    nc.sync.dma_start(
                        out=bin4.rearrange("p s t d -> p (s t d)"),
                        in_=bass.AP(
                            tensor=bias_c.ap().tensor,
                            offset=4 * g * L * D,
                            ap=[[3 * D, D], [L * D, 4], [1, 3 * D]],
                        ),
                    )
                    for s4 in range(4):
                        i_row = 4 * g + s4
                        bin_t = bin4[:, s4]
                        btp = btp_p.tile([D, L], bf16, tag="btp")
                        for t in range(3):
                            nc.tensor.transpose(
                                out=btp[:, t * D : (t + 1) * D],
                                in_=bin_t[:, t, :],
                                identity=id_bf,
                            )
                        bts = bt_p.tile([D, L], bf16, tag="bts")
                        nc.scalar.activation(out=bts, in_=btp, func=Identity)
                        bpp = bps_p.tile([D, 3, H], f32, tag="bps")
                        for t in range(3):
                            nc.tensor.matmul(
                                bpp[:, t, :],
                                bts[:, t * D : (t + 1) * D],
                                wb_sb,
                                start=True,
                                stop=True,
                            )
                        nc.vector.tensor_copy(
                            out=b_st[:, :, :, i_row], in_=bpp
                        )
                gate_rows(ph2, "a_", 0, 24)

            # ---------------- softmax on the i-shard ----------------
            with ExitStack() as ph3:
                sm_p = ph3.enter_context(tc.tile_pool(name="sm", bufs=1))
                smp_p = ph3.enter_context(
                    tc.tile_pool(name="smp", bufs=2, space="PSUM")
                )
                rs_sb = sm_p.tile([R, H, L], bf16)
                b2_sb = sm_p.tile([R, H, 3, D], bf16)
                nc.sync.dma_start(
                    out=rs_sb.rearrange("i h l -> i (h l)"),
                    in_=rs_out.ap().rearrange("i h l -> i (h l)"),
                )
                for h in range(H):
                    for jc in range(3):
                        btp2 = smp_p.tile([R, D], bf16, tag="btp2")
                        nc.tensor.transpose(
                            out=btp2, in_=b_st[:, jc, h, :], identity=id_bf
                        )
                        nc.vector.tensor_copy(out=b2_sb[:, h, jc, :], in_=btp2)
                ex_in = sm_p.tile([R, H, L], f32)
                nc.vector.tensor_add(
                    out=ex_in,
                    in0=rs_sb,
                    in1=b2_sb.rearrange("i h t d -> i h (t d)"),
                )
                exp_sb = sm_p.tile([R, H, L], f32)
                sums = sm_p.tile([R, H], f32)
                for h in range(H):
                    nc.scalar.activation(
                        out=exp_sb[:, h, :], in_=ex_in[:, h, :], func=Exp,
                        accum_out=sums[:, h : h + 1],
                    )
                rsum = sm_p.tile([R, H], f32)
                nc.vector.reciprocal(out=rsum, in_=sums)
                attn_sb = sm_p.tile([R, H, L], bf16)
                for h in range(H):
                    nc.vector.tensor_scalar(
                        out=attn_sb[:, h, :],
                        in0=exp_sb[:, h, :],
                        scalar1=rsum[:, h : h + 1],
                        scalar2=None,
                        op0=mult,
                    )
                nc.sync.dma_start(out=attn_bounce[:, :, :], in_=attn_sb)

            # ---------------- collective 2: AllGather attn ----------------
            nc.gpsimd.collective_compute(
                "AllGather",
                mybir.AluOpType.bypass,
                replica_groups=groups,
                ins=[attn_bounce.ap().opt()],
                outs=[attn_full.ap().opt()],
            )

            # ---------------- gate second half (overlaps AllGather) --------
            with ExitStack() as ph2b:
                gate_rows(ph2b, "b_", 24, R)

            # ---------------- phase 3: attn^T, AV, gate, Wo ----------------
            tc.strict_bb_all_engine_barrier()
            with ExitStack() as ph4:
                at_in_p = ph4.enter_context(tc.tile_pool(name="atin", bufs=3))
                at_ps_p = ph4.enter_context(
                    tc.tile_pool(name="atps", bufs=2, space="PSUM")
                )
                at_st = ph4.enter_context(tc.tile_pool(name="atst", bufs=1))
                attnT = at_st.tile([D, H, 3, L], bf16)  # [j, h, jc, x]
                at_in = [at_in_p.tile([D, H, L], bf16, tag="atin", name=f"at_in_{i}") for i in range(3)]
                for ic in range(3):
                    asrc = bass.AP(
                        tensor=attn_full.ap().tensor,
                        offset=ic * H * L,
                        ap=[[3 * H * L, D], [1, H * L]],
                    )
                    nc.sync.dma_start(
                        out=at_in[ic].rearrange("p h l -> p (h l)"), in_=asrc
                    )
                for h in range(H):
                    for jc in range(3):
                        pt = at_ps_p.tile([D, L], bf16, tag="atps")
                        for ic in range(3):
                            nc.tensor.transpose(
                                out=pt[:, ic * D : (ic + 1) * D],
                                in_=at_in[ic][:, h, jc * D : (jc + 1) * D],
                                identity=id_bf,
                            )
                        nc.scalar.activation(
                            out=attnT[:, h, jc, :], in_=pt, func=Identity
                        )

                av_p = ph4.enter_context(
                    tc.tile_pool(name="av", bufs=3, space="PSUM")
                )
                gt_p = ph4.enter_context(tc.tile_pool(name="gt", bufs=4))
                wo_ps = ph4.enter_context(
                    tc.tile_pool(name="wops", bufs=3, space="PSUM")
                )
                os_p = ph4.enter_context(tc.tile_pool(name="osb", bufs=4))
                for y in range(R):
                    pav = av_p.tile([D, L], f32, tag="av")
                    for h in range(H):
                        for jc in range(3):
                            nc.tensor.matmul(
                                pav[h * DH : (h + 1) * DH, :],
                                v_st[:, jc, y, h * DH : (h + 1) * DH],
                                attnT[:, h, jc, :],
                                start=(jc == 0),
                                stop=(jc == 2),
                                tile_position=(0, h * DH),
                            )
                    gated = gt_p.tile([D, L], bf16, tag="gt")
                    nc.vector.scalar_tensor_tensor(
                        out=gated,
                        in0=pav,
                        scalar=cv_sb,
                        in1=g_st[:, y, :],
                        op0=add,
                        op1=mult,
                    )
                    if y % 4 == 0:
                        osb4 = os_p.tile([D, 4, 3, D], f32, tag="osb",
                                         name=f"osb4_{y}")
                    # bo enters as a K=1 rank-1 matmul that seeds the psum
                    # accumulator; the three Wo matmuls accumulate on top.
                    pwo = wo_ps.tile([D, 3, D], f32, tag="wops")
                    nc.tensor.matmul(
                        pwo.rearrange("p a b -> p (a b)"),
                        ones_row,
                        bo3_row,
                        start=True,
                        stop=False,
                        skip_group_check=True,
                    )
                    for xc in range(3):
                        nc.tensor.matmul(
                            pwo[:, xc, :],
                            gated[:, xc * D : (xc + 1) * D],
                            wo_sb,
                            start=False,
                            stop=(xc == 2),
                            skip_group_check=True,
                        )
                    nc.scalar.activation(
                        out=osb4[:, y % 4], in_=pwo, func=Identity
                    )
                    if y % 4 == 3:
                        nc.sync.dma_start(
                            out=bass.AP(
                                tensor=out_p.ap().tensor,
                                offset=(y - 3) * L * D,
                                ap=[[3 * D, D], [L * D, 4], [1, 3 * D]],
                            ),
                            in_=osb4.rearrange("p s t d -> p (s t d)"),
                        )

    nc.compile()
    return nc


def _prep_host(inputs):
    """Host-side: shard inputs, fold LN scale/bias + constants into weights."""
    f32 = np.float32
    bf = ml_dtypes.bfloat16
    pair = np.ascontiguousarray(np.asarray(inputs["pair"], f32)[0])
    bias = np.ascontiguousarray(np.asarray(inputs["bias"], f32)[0])
    ln_scale = np.asarray(inputs["ln_scale"], f32)
    ln_bias = np.asarray(inputs["ln_bias"], f32)
    Wq = np.asarray(inputs["Wq"], f32)
    Wk = np.asarray(inputs["Wk"], f32)
    Wv = np.asarray(inputs["Wv"], f32)
    Wb = np.asarray(inputs["Wb"], f32)
    Wg = np.asarray(inputs["Wg"], f32)
    bg = np.asarray(inputs["bg"], f32)
    Wo = np.asarray(inputs["Wo"], f32)
    bo = np.asarray(inputs["bo"], f32)

    Wq_eff = Wq * ln_scale[None, :] * SCALING
    Wk_eff = Wk * ln_scale[None, :] * KSCALE
    cq = (Wq @ ln_bias) * SCALING
    ck = (Wk @ ln_bias) * KSCALE

    # blocked (s,dh) packing: q/k strip for row-in-group s lands on psum
    # partitions [32s, 32s+32); per-head weight block is just Wq_eff.T
    cq_sh = np.zeros((H, D), f32)
    ck_sh = np.zeros((H, D), f32)
    for h in range(H):
        for s in range(4):
            for kk in range(DH):
                cq_sh[h, s * DH + kk] = cq[h * DH + kk]
                ck_sh[h, s * DH + kk] = ck[h * DH + kk]

    pair_bf = pair.astype(bf)
    bias_bf = bias.astype(bf)

    shared = {
        "wq_t": Wq_eff.T.astype(bf).copy(),
        "wk_t": Wk_eff.T.astype(bf).copy(),
        "wv_t": (Wv * ln_scale[None, :]).T.astype(bf).copy(),
        "wg_t": (Wg * ln_scale[None, :]).T.astype(bf).copy(),
        "wo_t": Wo.T.astype(bf).copy(),
        "wb_t": Wb.T.astype(bf).copy(),
        "cq_sh": cq_sh,
        "ck_sh": ck_sh,
        "cg_v": (Wg @ ln_bias + bg).astype(f32),
        "cv_v": (Wv @ ln_bias).astype(f32),
        "bo3_v": np.tile(bo, 3).astype(bf),
    }
    in_maps = []
    for c in range(NCORES):
        sl = slice(c * R, (c + 1) * R)
        m = dict(shared)
        m["x_qkv"] = np.ascontiguousarray(pair_bf[:, sl, :].transpose(1, 0, 2))
        m["x_gate"] = np.ascontiguousarray(pair_bf[sl, :, :])
        m["bias_c"] = np.ascontiguousarray(bias_bf[sl, :, :])
        in_maps.append(m)
    return in_maps


def kernel(**inputs):
    import os
    from concourse.bass_utils import run_bass_kernel_spmd

    in_maps = _prep_host(inputs)
    if "nc" not in _CACHE:
        _CACHE["nc"] = _build_graph()
    nc = _CACHE["nc"]
    kw = {}
    if os.environ.get("BAX_TRACE"):
        kw = dict(trace=True, tmpdir=os.environ.get("BAX_TRACE_DIR") or None)
    res = run_bass_kernel_spmd(nc, in_maps, list(range(NCORES)), **kw)
    _CACHE["last_result"] = res
    out = np.zeros((1, L, L, D), np.float32)
    for c in range(NCORES):
        out[0, c * R : (c + 1) * R, :, :] = res.results[c]["out"]
    return out


if __name__ == "__main__":
    nc = _build_graph()
    print("graph built ok")
